# revision 50
# baseline (speedup 1.0000x reference)
"""Causal single-head attention layer on 8 TRN2 NeuronCores.

Problem: X[4,2048,1024]; Q/K/V = X@W+b; scores = Q@K^T (no 1/sqrt(d));
causal mask; softmax; out = P@V.

Sharding: 2 cores per batch. Each core owns 8 query tiles (128 rows) of
its batch, folded for causal load balance:
  core h=0 -> global q-tiles (0,3,4,7,8,11,12,15)
  core h=1 -> global q-tiles (1,2,5,6,9,10,13,14)
Slot s on either core has causal extent <= 2s+2 k-tiles, so ONE uniform
program runs on all 8 cores; the exact causal boundary is a host-supplied
0/1 mask over the last two k-tiles of each slot.

Math restructuring:
  scores = (XqWq+bq)(XkWk+bk)^T
         = Xq G Xk^T + [q-only term] + w[k] + [const],  G = Wq Wk^T (host)
  q-only and const terms cancel in softmax; w[k] = Xk @ (Wk bq) (host)
  rides the per-partition bias slot of the Exp activation.
  V projection is REASSOCIATED past the attention matmul:
      out = (E^T (Xk Wv)) / rowsum + bv  =  ((E^T Xk) Wv) / rowsum + bv
  Each core applies Wv only to its 8 q-tiles' T = E^T Xk ([1024, D])
  instead of projecting all 16 k-tiles' V ([2048, D]); this both halves
  the projection cost and removes the V-duplication across the two cores
  sharing a batch.  Per-core PE work drops ~348k -> ~283k cycles.

On-device phases (contraction always on partitions):
  1. Qg:      Qg^T[d2,q] = sum_d1 G[d1,d2] Xq^T[d1,q]      (fp32r)
  2. scores:  scores^T[k,q] accumulate fp32 in PSUM from xkt/qgt;
              E = exp(scores^T + w[k]) in bf16 (no max-subtraction:
              |scores| <= ~60 stays in range).
  3. T^T:     per slot, 8 chains (one per d-tile):
              T^T[d,q] = sum_k Xn[k,d]-stationary @ E[k,q]-moving (bf16)
  4. U:       U[q,d2] = sum_d Tt_sb[d,q]-stationary @ Wv[d,d2]-moving;
              out = (U * 1/rowsum) + bv fused via scalar_tensor_tensor,
              stored bf16 (host upcasts).
  Row sums via per-slot matmul chain with a ones vector.

DMA: big 2D transfers (host pre-blocks G/Xq/Xn/Wv so each transfer is a
plain 2D slice with >=2KB contiguous rows); HWDGE ~650ns/transfer makes
transfer COUNT matter.  Order paces the compute: g0,xq -> g1-7 -> xk
(k-tiles 0-7) -> small -> [WAR on xq space] xn0-1, wv, xn2-7 -> xk
(k-tiles 8-15) -> xn8-15.
"""

import numpy as np
import ml_dtypes

import concourse.bass as bass  # noqa: F401
import concourse.mybir as mybir
from concourse import bacc
from concourse.bass_utils import run_bass_kernel_spmd
from concourse.tile import TileContext

F32 = mybir.dt.float32
F32R = mybir.dt.float32r
BF16 = mybir.dt.bfloat16
EXP = mybir.ActivationFunctionType.Exp
MUL = mybir.AluOpType.mult
ADD = mybir.AluOpType.add

B, S, D = 4, 2048, 1024
P = 128
DT = D // P          # 8 d-tiles
QT = 8               # q-tile slots per core
KT = S // P          # 16 k-tiles
EXT = [2 * s + 2 for s in range(QT)]   # uniform per-slot k-extent
BLK = [(0, 4, 8), (4, 8, 16)]          # (slot_lo, slot_hi, block k-extent)

QTS = {0: [0, 3, 4, 7, 8, 11, 12, 15], 1: [1, 2, 5, 6, 9, 10, 13, 14]}

_CACHE = {}


def _build(reps=1):
    nc = bacc.Bacc("TRN2", target_bir_lowering=False, debug=False, num_devices=8)
    # host-preblocked layouts (see make_in_maps)
    xqt = nc.declare_dram_parameter("xqt", [P, DT * (QT * P)], F32R, isOutput=False)
    xkt = nc.declare_dram_parameter("xkt", [P, KT * D], F32R, isOutput=False)
    g = nc.declare_dram_parameter("g", [DT, P, D], F32R, isOutput=False)
    xn = nc.declare_dram_parameter("xn", [P, KT * D], BF16, isOutput=False)
    wv = nc.declare_dram_parameter("wv", [P, DT * D], BF16, isOutput=False)
    wb = nc.declare_dram_parameter("wb", [P, KT], F32, isOutput=False)
    bvp = nc.declare_dram_parameter("bvp", [P, D], F32, isOutput=False)
    msk = nc.declare_dram_parameter("msk", [P, QT * 2 * P], BF16, isOutput=False)
    y = nc.declare_dram_parameter("y", [QT * P, D], BF16, isOutput=True)

    with TileContext(nc) as tc:
      for _rep in range(reps):
        with tc.tile_pool(name="persist", bufs=1) as pp:
            # ---- persistent tiles ----
            # kt-major: one 0.5MB DMA delivers one k-tile's full [d, k] slab,
            # so each scores chain can start as soon as ITS k-tile lands
            xk_sb = pp.tile([P, KT * D], F32R, tag="xk", name="xk_sb")
            qg_sb = [pp.tile([P, QT * P], F32R, tag=f"qg{i}", name=f"qg{i}") for i in range(DT)]
            wb_sb = pp.tile([P, KT], F32, tag="wb")
            bv_sb = pp.tile([P, D], F32, tag="bv")
            mask_sb = pp.tile([P, QT * 2 * P], BF16, tag="mask")
            ones_sb = pp.tile([P, 1], BF16, tag="ones")

            # ---- Phase Qg (scoped: xq/g space is reused for xn/wv later) ----
            with tc.tile_pool(name="psproj", bufs=6, space="PSUM") as ps:
                with tc.tile_pool(name="qgpool", bufs=2) as qp:
                    xq_sb = qp.tile([P, DT * QT * P], F32R, bufs=1, tag="xq")
                    g_all = {}

                    def _fetch_g(do):
                        g_all[do] = qp.tile([P, D], F32R, tag="gdo", bufs=4,
                                            name=f"g{do}")
                        nc.sync.dma_start(out=g_all[do][:], in_=g[do, :, :])

                    # small leading chunks so the first Qg chain starts early;
                    # the first matmul's two inputs are the first two
                    # transfers on the wire.
                    g_all[0] = qp.tile([P, D], F32R, tag="gdo", bufs=4, name="g0")
                    nc.sync.dma_start(out=g_all[0][:, 0:128], in_=g[0, :, 0:128])
                    nc.sync.dma_start(out=xq_sb[:, 0:512], in_=xqt[:, 0:512])
                    nc.sync.dma_start(out=g_all[0][:, 128:1024],
                                      in_=g[0, :, 128:1024])
                    nc.sync.dma_start(out=xq_sb[:, 512:1024],
                                      in_=xqt[:, 512:1024])
                    _fetch_g(1)
                    for dd in range(1, DT):
                        nc.sync.dma_start(
                            out=xq_sb[:, dd * 1024:(dd + 1) * 1024],
                            in_=xqt[:, dd * 1024:(dd + 1) * 1024])
                        if dd == 2:
                            _fetch_g(2)
                    for do in range(3, DT):
                        _fetch_g(do)

                    def _qg_copy(do, c, pq):
                        if (do + c) % 2 == 0:
                            nc.vector.tensor_copy(
                                qg_sb[do][:, c * 512:(c + 1) * 512], pq[:])
                        else:
                            nc.scalar.copy(
                                qg_sb[do][:, c * 512:(c + 1) * 512], pq[:])

                    # do=0/1 interleaved at dd granularity (each arriving xq
                    # tile feeds 4 matmuls), with do=2's chain TRAILING by 3
                    # dd-steps: its matmuls read xq tiles that already landed,
                    # so they never park at the in-order wait-queue head, and
                    # the xq streaming window feeds 6 matmuls per tile.
                    def _ilv_mm(do, c, dd):
                        nc.tensor.matmul(
                            pq01[(do, c)][:],
                            g_all[do][:, dd * P:(dd + 1) * P],
                            xq_sb[:, dd * 1024 + c * 512:
                                  dd * 1024 + (c + 1) * 512],
                            start=(dd == 0), stop=(dd == DT - 1),
                        )

                    TRAIL = 3
                    pq01 = {(do, c): ps.tile([P, 512], F32, tag="pq", bufs=6,
                                             name=f"pq{do}{c}")
                            for do in (0, 1, 2) for c in (0, 1)}
                    for dd in range(DT + TRAIL):
                        if dd < DT:
                            for do in (0, 1):
                                for c in range(2):
                                    _ilv_mm(do, c, dd)
                        if dd >= TRAIL:
                            for c in range(2):
                                _ilv_mm(2, c, dd - TRAIL)
                    for do in (0, 1, 2):
                        for c in range(2):
                            _qg_copy(do, c, pq01[(do, c)])

                    for do in range(3, DT):
                        for c in range(2):
                            pq = ps.tile([P, 512], F32, tag="pq", bufs=6)
                            for dd in range(DT):
                                nc.tensor.matmul(
                                    pq[:],
                                    g_all[do][:, dd * P:(dd + 1) * P],
                                    xq_sb[:, dd * 1024 + c * 512:
                                          dd * 1024 + (c + 1) * 512],
                                    start=(dd == 0), stop=(dd == DT - 1),
                                )
                            _qg_copy(do, c, pq)

                    # scores inputs stream behind the Qg compute (kt-major)
                    for kt in range(8):
                        nc.sync.dma_start(
                            out=xk_sb[:, kt * 1024:(kt + 1) * 1024],
                            in_=xkt[:, kt * 1024:(kt + 1) * 1024])
                    nc.sync.dma_start(out=wb_sb[:], in_=wb[:])
                    nc.sync.dma_start(out=bv_sb[:], in_=bvp[:])
                    nc.sync.dma_start(out=mask_sb[:], in_=msk[:])
                    nc.gpsimd.memset(ones_sb[:], 1.0)

            # xn/wv reuse the xq/g SBUF space (WAR-gated on Qg's last read)
            with (
                tc.tile_pool(name="xnwv", bufs=1) as xp,
                tc.tile_pool(name="estage", bufs=24) as ep,
                tc.tile_pool(name="ttstage", bufs=2) as tp,
                tc.tile_pool(name="ostage", bufs=2) as op,
                tc.tile_pool(name="small", bufs=4) as sp,
                # open order = PSUM bank order: tt/pa/pb land on the banks
                # the Qg-phase psum used (their first use is well after that
                # pool closes), so the first scores chain has no WAR stall.
                tc.tile_pool(name="pstt", bufs=1, space="PSUM") as ps_t,
                tc.tile_pool(name="psa", bufs=1, space="PSUM") as ps_a,
                tc.tile_pool(name="psb", bufs=1, space="PSUM") as ps_b,
                tc.tile_pool(name="psm", bufs=1, space="PSUM") as ps_m,
                tc.tile_pool(name="pssc", bufs=3, space="PSUM") as ps_s,
            ):
                xn_sb = xp.tile([P, KT * 1024], BF16, tag="xn")
                wv_sb = xp.tile([P, DT * 1024], BF16, tag="wv")
                # priority order: T/U of block A needs wv + xn k0-7 first
                for half in range(2):
                    nc.sync.dma_start(
                        out=wv_sb[:, half * 4096:(half + 1) * 4096],
                        in_=wv[:, half * 4096:(half + 1) * 4096])
                nc.sync.dma_start(out=xn_sb[:, 0:2048], in_=xn[:, 0:2048])
                for k2 in range(1, 4):
                    nc.sync.dma_start(
                        out=xn_sb[:, k2 * 2048:(k2 + 1) * 2048],
                        in_=xn[:, k2 * 2048:(k2 + 1) * 2048])
                for kt in range(8, KT):
                    nc.sync.dma_start(
                        out=xk_sb[:, kt * 1024:(kt + 1) * 1024],
                        in_=xkt[:, kt * 1024:(kt + 1) * 1024])
                for k2 in range(4, 8):
                    nc.sync.dma_start(
                        out=xn_sb[:, k2 * 2048:(k2 + 1) * 2048],
                        in_=xn[:, k2 * 2048:(k2 + 1) * 2048])

                # ---- Attention ----
                for (s0, s1, bext) in BLK:
                    q0 = s0 * P
                    e_tiles = []
                    e_offs = []
                    for kt in range(bext):
                        # slots below ls_min never read k-tile kt (causal):
                        # narrow the moving dim, keeping N >= 256 so fp32r
                        # stays at 1 cycle/row.
                        ls_min = max(0, kt // 2)
                        off = min(max(0, (ls_min - s0)) * P, 256)
                        n = 512 - off
                        pscore = ps_s.tile([P, 512], F32, tag="sc")
                        for dd in range(DT):
                            nc.tensor.matmul(
                                pscore[:, 0:n],
                                xk_sb[:, kt * 1024 + dd * P:
                                      kt * 1024 + (dd + 1) * P],
                                qg_sb[dd][:, q0 + off:q0 + 512],
                                start=(dd == 0), stop=(dd == DT - 1),
                            )
                        et = ep.tile([P, 512], BF16, tag="E")
                        # E = exp(scores^T + w[k])  (w rides the bias slot)
                        nc.scalar.activation(et[:, 0:n], pscore[:, 0:n], EXP,
                                             bias=wb_sb[:, kt:kt + 1])
                        e_tiles.append(et)
                        e_offs.append(off)
                        # causal boundary mask (hoisted: each k-tile kt is the
                        # boundary of slot ls = kt//2; apply right after exp so
                        # no slot's PE work ever waits on a mask-mul)
                        bs = kt // 2   # slot (global) whose boundary is kt
                        if s0 <= bs < s1:
                            j = kt % 2
                            lo = (bs - s0) * P - off
                            eng = nc.vector if kt % 2 == 0 else nc.gpsimd
                            eng.tensor_mul(
                                et[:, lo:lo + P],
                                et[:, lo:lo + P],
                                mask_sb[:, (2 * bs + j) * P:(2 * bs + j + 1) * P],
                            )

                    for ls in range(s0, s1):
                        lq = (ls - s0) * P
                        ext = EXT[ls]

                        def esl(kt):
                            lo = lq - e_offs[kt]
                            return e_tiles[kt][:, lo:lo + P]

                        # T^T[d,q] chains: one per d-tile, in two half-tiles
                        # (separate Tile objects: deps are tile-granular, so a
                        # half's single copy never WAR-stalls the other half's
                        # chains).  copy-A overlaps chains 4-7; copy-B overlaps
                        # U's dd 0-3.
                        ptt = [ps_t.tile([P, 4 * P], F32, tag=f"tt{h}",
                                         name=f"ptt{h}") for h in range(2)]
                        tt_sb = [tp.tile([P, 4 * P], BF16, tag=f"tts{h}",
                                         name=f"tts{h}") for h in range(2)]
                        for h in range(2):
                            for d4 in range(4):
                                dd = h * 4 + d4
                                for kt in range(ext):
                                    nc.tensor.matmul(
                                        ptt[h][:, d4 * P:(d4 + 1) * P],
                                        xn_sb[:, kt * 1024 + dd * P:
                                              kt * 1024 + (dd + 1) * P],
                                        esl(kt),
                                        start=(kt == 0), stop=(kt == ext - 1),
                                    )
                            if h == 0:
                                nc.vector.tensor_copy(tt_sb[0][:], ptt[0][:])
                            else:
                                # on the critical path to U dd=4: split across
                                # DVE + Act so both halves land in ~450ns
                                nc.scalar.copy(tt_sb[1][:, 0:256],
                                               ptt[1][:, 0:256])
                                nc.vector.tensor_copy(tt_sb[1][:, 256:512],
                                                      ptt[1][:, 256:512])

                        # row sums AFTER T^T: their ps_m WAR (previous slot's
                        # reciprocal) is long past, so these tiny waiting
                        # matmuls never clog the PE wait queue (depth 4) and
                        # stall ready T^T work.
                        pm = ps_m.tile([P, 1], F32, tag="pm")
                        for kt in range(ext):
                            nc.tensor.matmul(pm[:], esl(kt), ones_sb[:],
                                             start=(kt == 0), stop=(kt == ext - 1))
                        rc = sp.tile([P, 1], F32, tag="rc")
                        nc.vector.reciprocal(rc[:], pm[:])

                        # pa chain fully first: its scale+bias+DMA overlap the
                        # pb chain, shortening the exposed tail after the last
                        # PE matmul of the kernel.
                        pa = ps_a.tile([P, 512], F32, tag="pa")
                        pb = ps_b.tile([P, 512], F32, tag="pb")
                        ot = op.tile([P, D], BF16, tag="ot")
                        for dd in range(DT):
                            nc.tensor.matmul(pa[:], tt_sb[dd // 4][:, (dd % 4) * P:
                                             (dd % 4 + 1) * P],
                                             wv_sb[:, dd * 1024:dd * 1024 + 512],
                                             start=(dd == 0), stop=(dd == DT - 1))
                        nc.vector.scalar_tensor_tensor(
                            ot[:, 0:512], pa[:], rc[:], bv_sb[:, 0:512],
                            op0=MUL, op1=ADD)
                        nc.sync.dma_start(out=y[ls * P:(ls + 1) * P, 0:512],
                                          in_=ot[:, 0:512])
                        for dd in range(DT):
                            nc.tensor.matmul(pb[:], tt_sb[dd // 4][:, (dd % 4) * P:
                                             (dd % 4 + 1) * P],
                                             wv_sb[:, dd * 1024 + 512:
                                                   (dd + 1) * 1024],
                                             start=(dd == 0), stop=(dd == DT - 1))
                        nc.vector.scalar_tensor_tensor(
                            ot[:, 512:1024], pb[:], rc[:],
                            bv_sb[:, 512:1024], op0=MUL, op1=ADD)
                        nc.sync.dma_start(
                            out=y[ls * P:(ls + 1) * P, 512:1024],
                            in_=ot[:, 512:1024])

    nc.compile()
    return nc


def _get_nc():
    if "nc" not in _CACHE:
        _CACHE["nc"] = _build()
    return _CACHE["nc"]


def make_in_maps(X, Wq, bq, Wk, bk, Wv, bv):
    X = np.asarray(X, np.float32)
    Wq = np.asarray(Wq, np.float32)
    Wk = np.asarray(Wk, np.float32)
    Wv = np.asarray(Wv, np.float32)
    bq = np.asarray(bq, np.float32)
    bv = np.asarray(bv, np.float32)

    G = Wq @ Wk.T                                # [D, D]
    wkbq = Wk @ bq                               # [D]
    bvp = np.ascontiguousarray(np.broadcast_to(bv[None, :], (P, D)))
    # g[do, p, dd*128+c] = G[dd*128+p, do*128+c]
    g_blk = np.ascontiguousarray(
        G.reshape(DT, P, DT, P).transpose(2, 1, 0, 3).reshape(DT, P, D))
    # wv[p, dd*1024+c] = Wv[dd*128+p, c]
    wv_blk = np.ascontiguousarray(
        Wv.astype(ml_dtypes.bfloat16).reshape(DT, P, D)
        .transpose(1, 0, 2).reshape(P, DT * D))

    masks = {}
    for h in (0, 1):
        m = np.zeros((QT, 2 * P, P), np.float32)
        for s in range(QT):
            qt = QTS[h][s]
            kk = (2 * s) * P + np.arange(2 * P)[:, None]
            qq = qt * P + np.arange(P)[None, :]
            m[s] = (kk <= qq)
        # msk2[p, t*128+c] = m.reshape(16,128,128)[t, p, c]
        masks[h] = np.ascontiguousarray(
            m.reshape(QT * 2, P, P).transpose(1, 0, 2).reshape(P, QT * 2 * P)
        ).astype(ml_dtypes.bfloat16)

    in_maps = []
    for c in range(8):
        b, h = divmod(c, 2)
        Xb = X[b]
        # xkt[p, kt*1024 + dd*128 + c] = Xb.T[dd*128+p, kt*128+c]
        xkt = np.ascontiguousarray(
            Xb.T.reshape(DT, P, KT, P).transpose(1, 2, 0, 3)
            .reshape(P, KT * D))
        xq_rows = np.concatenate(
            [Xb[qt * P:(qt + 1) * P] for qt in QTS[h]], axis=0)
        # xqt[p, dd*1024+q] = xq_rows.T[dd*128+p, q]
        xqt = np.ascontiguousarray(
            xq_rows.T.reshape(DT, P, QT * P).transpose(1, 0, 2)
            .reshape(P, DT * QT * P))
        # xn[p, kt*1024+d] = Xb[kt*128+p, d]  (natural layout, bf16)
        xn = np.ascontiguousarray(
            Xb.astype(ml_dtypes.bfloat16).reshape(KT, P, D)
            .transpose(1, 0, 2).reshape(P, KT * D))
        w = Xb @ wkbq                             # [S] additive k-bias
        wbp = np.ascontiguousarray(w.reshape(KT, P).T)   # [P, KT]
        in_maps.append({
            "xqt": xqt, "xkt": xkt, "g": g_blk, "xn": xn, "wv": wv_blk,
            "wb": wbp, "bvp": bvp, "msk": masks[h],
        })
    return in_maps


def assemble(results):
    Y = np.empty((B, S, D), np.float32)
    for c in range(8):
        b, h = divmod(c, 2)
        yc = np.asarray(results[c]["y"], dtype=np.float32)
        for s in range(QT):
            qt = QTS[h][s]
            Y[b, qt * P:(qt + 1) * P, :] = yc[s * P:(s + 1) * P, :]
    return Y


def kernel(X, Wq, bq, Wk, bk, Wv, bv):
    nc = _get_nc()
    in_maps = make_in_maps(X, Wq, bq, Wk, bk, Wv, bv)
    res = run_bass_kernel_spmd(nc, in_maps, core_ids=list(range(8)))
    return assemble(res.results)


# revision 51
# speedup vs baseline: 1.0017x; 1.0017x over previous
"""Causal single-head attention layer on 8 TRN2 NeuronCores.

Problem: X[4,2048,1024]; Q/K/V = X@W+b; scores = Q@K^T (no 1/sqrt(d));
causal mask; softmax; out = P@V.

Sharding: 2 cores per batch. Each core owns 8 query tiles (128 rows) of
its batch, folded for causal load balance:
  core h=0 -> global q-tiles (0,3,4,7,8,11,12,15)
  core h=1 -> global q-tiles (1,2,5,6,9,10,13,14)
Slot s on either core has causal extent <= 2s+2 k-tiles, so ONE uniform
program runs on all 8 cores; the exact causal boundary is a host-supplied
0/1 mask over the last two k-tiles of each slot.

Math restructuring:
  scores = (XqWq+bq)(XkWk+bk)^T
         = Xq G Xk^T + [q-only term] + w[k] + [const],  G = Wq Wk^T (host)
  q-only and const terms cancel in softmax; w[k] = Xk @ (Wk bq) (host)
  rides the per-partition bias slot of the Exp activation.
  V projection is REASSOCIATED past the attention matmul:
      out = (E^T (Xk Wv)) / rowsum + bv  =  ((E^T Xk) Wv) / rowsum + bv
  Each core applies Wv only to its 8 q-tiles' T = E^T Xk ([1024, D])
  instead of projecting all 16 k-tiles' V ([2048, D]); this both halves
  the projection cost and removes the V-duplication across the two cores
  sharing a batch.  Per-core PE work drops ~348k -> ~283k cycles.

On-device phases (contraction always on partitions):
  1. Qg:      Qg^T[d2,q] = sum_d1 G[d1,d2] Xq^T[d1,q]      (fp32r)
  2. scores:  scores^T[k,q] accumulate fp32 in PSUM from xkt/qgt;
              E = exp(scores^T + w[k]) in bf16 (no max-subtraction:
              |scores| <= ~60 stays in range).
  3. T^T:     per slot, 8 chains (one per d-tile):
              T^T[d,q] = sum_k Xn[k,d]-stationary @ E[k,q]-moving (bf16)
  4. U:       U[q,d2] = sum_d Tt_sb[d,q]-stationary @ Wv[d,d2]-moving;
              out = (U * 1/rowsum) + bv fused via scalar_tensor_tensor,
              stored bf16 (host upcasts).
  Row sums via per-slot matmul chain with a ones vector.

DMA: big 2D transfers (host pre-blocks G/Xq/Xn/Wv so each transfer is a
plain 2D slice with >=2KB contiguous rows); HWDGE ~650ns/transfer makes
transfer COUNT matter.  Order paces the compute: g0,xq -> g1-7 -> xk
(k-tiles 0-7) -> small -> [WAR on xq space] xn0-1, wv, xn2-7 -> xk
(k-tiles 8-15) -> xn8-15.
"""

import numpy as np
import ml_dtypes

import concourse.bass as bass  # noqa: F401
import concourse.mybir as mybir
from concourse import bacc
from concourse.bass_utils import run_bass_kernel_spmd
from concourse.tile import TileContext

F32 = mybir.dt.float32
F32R = mybir.dt.float32r
BF16 = mybir.dt.bfloat16
EXP = mybir.ActivationFunctionType.Exp
MUL = mybir.AluOpType.mult
ADD = mybir.AluOpType.add

B, S, D = 4, 2048, 1024
P = 128
DT = D // P          # 8 d-tiles
QT = 8               # q-tile slots per core
KT = S // P          # 16 k-tiles
EXT = [2 * s + 2 for s in range(QT)]   # uniform per-slot k-extent
BLK = [(0, 4, 8), (4, 8, 16)]          # (slot_lo, slot_hi, block k-extent)

QTS = {0: [0, 3, 4, 7, 8, 11, 12, 15], 1: [1, 2, 5, 6, 9, 10, 13, 14]}

_CACHE = {}


def _build(reps=1):
    nc = bacc.Bacc("TRN2", target_bir_lowering=False, debug=False, num_devices=8)
    # host-preblocked layouts (see make_in_maps)
    xqt = nc.declare_dram_parameter("xqt", [P, DT * (QT * P)], F32R, isOutput=False)
    xkt = nc.declare_dram_parameter("xkt", [P, KT * D], F32R, isOutput=False)
    g = nc.declare_dram_parameter("g", [DT, P, D], F32R, isOutput=False)
    xn = nc.declare_dram_parameter("xn", [P, KT * D], BF16, isOutput=False)
    wv = nc.declare_dram_parameter("wv", [P, DT * D], BF16, isOutput=False)
    wb = nc.declare_dram_parameter("wb", [P, KT], F32, isOutput=False)
    bvp = nc.declare_dram_parameter("bvp", [P, D], F32, isOutput=False)
    msk = nc.declare_dram_parameter("msk", [P, QT * 2 * P], BF16, isOutput=False)
    y = nc.declare_dram_parameter("y", [QT * P, D], BF16, isOutput=True)

    with TileContext(nc) as tc:
      for _rep in range(reps):
        with tc.tile_pool(name="persist", bufs=1) as pp:
            # ---- persistent tiles ----
            # kt-major: one 0.5MB DMA delivers one k-tile's full [d, k] slab,
            # so each scores chain can start as soon as ITS k-tile lands
            xk_sb = pp.tile([P, KT * D], F32R, tag="xk", name="xk_sb")
            qg_sb = [pp.tile([P, QT * P], F32R, tag=f"qg{i}", name=f"qg{i}") for i in range(DT)]
            wb_sb = pp.tile([P, KT], F32, tag="wb")
            bv_sb = pp.tile([P, D], F32, tag="bv")
            mask_sb = pp.tile([P, QT * 2 * P], BF16, tag="mask")
            ones_sb = pp.tile([P, 1], BF16, tag="ones")

            # ---- Phase Qg (scoped: xq/g space is reused for xn/wv later) ----
            with tc.tile_pool(name="psproj", bufs=6, space="PSUM") as ps:
                with tc.tile_pool(name="qgpool", bufs=2) as qp:
                    xq_sb = qp.tile([P, DT * QT * P], F32R, bufs=1, tag="xq")
                    g_all = {}

                    def _fetch_g(do):
                        g_all[do] = qp.tile([P, D], F32R, tag="gdo", bufs=4,
                                            name=f"g{do}")
                        nc.sync.dma_start(out=g_all[do][:], in_=g[do, :, :])

                    # small leading chunks so the first Qg chain starts early;
                    # the first matmul's two inputs are the first two
                    # transfers on the wire.
                    g_all[0] = qp.tile([P, D], F32R, tag="gdo", bufs=4, name="g0")
                    nc.sync.dma_start(out=g_all[0][:, 0:128], in_=g[0, :, 0:128])
                    nc.sync.dma_start(out=xq_sb[:, 0:512], in_=xqt[:, 0:512])
                    nc.sync.dma_start(out=g_all[0][:, 128:1024],
                                      in_=g[0, :, 128:1024])
                    nc.sync.dma_start(out=xq_sb[:, 512:1024],
                                      in_=xqt[:, 512:1024])
                    _fetch_g(1)
                    for dd in range(1, DT):
                        nc.sync.dma_start(
                            out=xq_sb[:, dd * 1024:(dd + 1) * 1024],
                            in_=xqt[:, dd * 1024:(dd + 1) * 1024])
                        if dd == 2:
                            _fetch_g(2)
                    for do in range(3, DT):
                        _fetch_g(do)

                    def _qg_copy(do, c, pq):
                        if (do + c) % 2 == 0:
                            nc.vector.tensor_copy(
                                qg_sb[do][:, c * 512:(c + 1) * 512], pq[:])
                        else:
                            nc.scalar.copy(
                                qg_sb[do][:, c * 512:(c + 1) * 512], pq[:])

                    # do=0/1 interleaved at dd granularity (each arriving xq
                    # tile feeds 4 matmuls), with do=2's chain TRAILING by 3
                    # dd-steps: its matmuls read xq tiles that already landed,
                    # so they never park at the in-order wait-queue head, and
                    # the xq streaming window feeds 6 matmuls per tile.
                    def _ilv_mm(do, c, dd):
                        nc.tensor.matmul(
                            pq01[(do, c)][:],
                            g_all[do][:, dd * P:(dd + 1) * P],
                            xq_sb[:, dd * 1024 + c * 512:
                                  dd * 1024 + (c + 1) * 512],
                            start=(dd == 0), stop=(dd == DT - 1),
                        )

                    TRAIL = 3
                    pq01 = {(do, c): ps.tile([P, 512], F32, tag="pq", bufs=6,
                                             name=f"pq{do}{c}")
                            for do in (0, 1, 2) for c in (0, 1)}
                    for dd in range(DT + TRAIL):
                        if dd < DT:
                            for do in (0, 1):
                                for c in range(2):
                                    _ilv_mm(do, c, dd)
                        if dd >= TRAIL:
                            for c in range(2):
                                _ilv_mm(2, c, dd - TRAIL)
                    for do in (0, 1, 2):
                        for c in range(2):
                            _qg_copy(do, c, pq01[(do, c)])

                    for do in range(3, DT):
                        for c in range(2):
                            if do == DT - 1 and c == 1:
                                continue
                            pq = ps.tile([P, 512], F32, tag="pq", bufs=6)
                            for dd in range(DT):
                                nc.tensor.matmul(
                                    pq[:],
                                    g_all[do][:, dd * P:(dd + 1) * P],
                                    xq_sb[:, dd * 1024 + c * 512:
                                          dd * 1024 + (c + 1) * 512],
                                    start=(dd == 0), stop=(dd == DT - 1),
                                )
                            _qg_copy(do, c, pq)
                    # last chain in two half-width chains so its psum copies
                    # pipeline: the attention pools' open barrier waits on the
                    # final copy, which now lags the last matmul by only
                    # ~300ns instead of ~700ns.
                    for half in range(2):
                        lo = 512 + half * 256
                        pq = ps.tile([P, 512], F32, tag="pq", bufs=6)
                        for dd in range(DT):
                            nc.tensor.matmul(
                                pq[:, 0:256],
                                g_all[DT - 1][:, dd * P:(dd + 1) * P],
                                xq_sb[:, dd * 1024 + lo:dd * 1024 + lo + 256],
                                start=(dd == 0), stop=(dd == DT - 1),
                            )
                        if half == 0:
                            nc.vector.tensor_copy(
                                qg_sb[DT - 1][:, lo:lo + 256], pq[:, 0:256])
                        else:
                            nc.scalar.copy(
                                qg_sb[DT - 1][:, lo:lo + 256], pq[:, 0:256])

                    # scores inputs stream behind the Qg compute (kt-major)
                    for kt in range(8):
                        nc.sync.dma_start(
                            out=xk_sb[:, kt * 1024:(kt + 1) * 1024],
                            in_=xkt[:, kt * 1024:(kt + 1) * 1024])
                    nc.sync.dma_start(out=wb_sb[:], in_=wb[:])
                    nc.sync.dma_start(out=bv_sb[:], in_=bvp[:])
                    nc.sync.dma_start(out=mask_sb[:], in_=msk[:])
                    nc.gpsimd.memset(ones_sb[:], 1.0)

            # xn/wv reuse the xq/g SBUF space (WAR-gated on Qg's last read)
            with (
                tc.tile_pool(name="xnwv", bufs=1) as xp,
                tc.tile_pool(name="estage", bufs=24) as ep,
                tc.tile_pool(name="ttstage", bufs=2) as tp,
                tc.tile_pool(name="ostage", bufs=2) as op,
                tc.tile_pool(name="small", bufs=4) as sp,
                # open order = PSUM bank order: tt/pa/pb land on the banks
                # the Qg-phase psum used (their first use is well after that
                # pool closes), so the first scores chain has no WAR stall.
                tc.tile_pool(name="pstt", bufs=1, space="PSUM") as ps_t,
                tc.tile_pool(name="psa", bufs=1, space="PSUM") as ps_a,
                tc.tile_pool(name="psb", bufs=1, space="PSUM") as ps_b,
                tc.tile_pool(name="psm", bufs=1, space="PSUM") as ps_m,
                tc.tile_pool(name="pssc", bufs=3, space="PSUM") as ps_s,
            ):
                xn_sb = xp.tile([P, KT * 1024], BF16, tag="xn")
                wv_sb = xp.tile([P, DT * 1024], BF16, tag="wv")
                # priority order: T/U of block A needs wv + xn k0-7 first
                for half in range(2):
                    nc.sync.dma_start(
                        out=wv_sb[:, half * 4096:(half + 1) * 4096],
                        in_=wv[:, half * 4096:(half + 1) * 4096])
                nc.sync.dma_start(out=xn_sb[:, 0:2048], in_=xn[:, 0:2048])
                for k2 in range(1, 4):
                    nc.sync.dma_start(
                        out=xn_sb[:, k2 * 2048:(k2 + 1) * 2048],
                        in_=xn[:, k2 * 2048:(k2 + 1) * 2048])
                for kt in range(8, KT):
                    nc.sync.dma_start(
                        out=xk_sb[:, kt * 1024:(kt + 1) * 1024],
                        in_=xkt[:, kt * 1024:(kt + 1) * 1024])
                for k2 in range(4, 8):
                    nc.sync.dma_start(
                        out=xn_sb[:, k2 * 2048:(k2 + 1) * 2048],
                        in_=xn[:, k2 * 2048:(k2 + 1) * 2048])

                # ---- Attention ----
                for (s0, s1, bext) in BLK:
                    q0 = s0 * P
                    e_tiles = []
                    e_offs = []
                    for kt in range(bext):
                        # slots below ls_min never read k-tile kt (causal):
                        # narrow the moving dim, keeping N >= 256 so fp32r
                        # stays at 1 cycle/row.
                        ls_min = max(0, kt // 2)
                        off = min(max(0, (ls_min - s0)) * P, 256)
                        n = 512 - off
                        pscore = ps_s.tile([P, 512], F32, tag="sc")
                        for dd in range(DT):
                            nc.tensor.matmul(
                                pscore[:, 0:n],
                                xk_sb[:, kt * 1024 + dd * P:
                                      kt * 1024 + (dd + 1) * P],
                                qg_sb[dd][:, q0 + off:q0 + 512],
                                start=(dd == 0), stop=(dd == DT - 1),
                            )
                        et = ep.tile([P, 512], BF16, tag="E")
                        # E = exp(scores^T + w[k])  (w rides the bias slot)
                        nc.scalar.activation(et[:, 0:n], pscore[:, 0:n], EXP,
                                             bias=wb_sb[:, kt:kt + 1])
                        e_tiles.append(et)
                        e_offs.append(off)
                        # causal boundary mask (hoisted: each k-tile kt is the
                        # boundary of slot ls = kt//2; apply right after exp so
                        # no slot's PE work ever waits on a mask-mul)
                        bs = kt // 2   # slot (global) whose boundary is kt
                        if s0 <= bs < s1:
                            j = kt % 2
                            lo = (bs - s0) * P - off
                            eng = nc.vector if kt % 2 == 0 else nc.gpsimd
                            eng.tensor_mul(
                                et[:, lo:lo + P],
                                et[:, lo:lo + P],
                                mask_sb[:, (2 * bs + j) * P:(2 * bs + j + 1) * P],
                            )

                    for ls in range(s0, s1):
                        lq = (ls - s0) * P
                        ext = EXT[ls]

                        def esl(kt):
                            lo = lq - e_offs[kt]
                            return e_tiles[kt][:, lo:lo + P]

                        # T^T[d,q] chains: one per d-tile, in two half-tiles
                        # (separate Tile objects: deps are tile-granular, so a
                        # half's single copy never WAR-stalls the other half's
                        # chains).  copy-A overlaps chains 4-7; copy-B overlaps
                        # U's dd 0-3.
                        ptt = [ps_t.tile([P, 4 * P], F32, tag=f"tt{h}",
                                         name=f"ptt{h}") for h in range(2)]
                        tt_sb = [tp.tile([P, 4 * P], BF16, tag=f"tts{h}",
                                         name=f"tts{h}") for h in range(2)]
                        for h in range(2):
                            for d4 in range(4):
                                dd = h * 4 + d4
                                for kt in range(ext):
                                    nc.tensor.matmul(
                                        ptt[h][:, d4 * P:(d4 + 1) * P],
                                        xn_sb[:, kt * 1024 + dd * P:
                                              kt * 1024 + (dd + 1) * P],
                                        esl(kt),
                                        start=(kt == 0), stop=(kt == ext - 1),
                                    )
                            if h == 0:
                                nc.vector.tensor_copy(tt_sb[0][:], ptt[0][:])
                            else:
                                # on the critical path to U dd=4: split across
                                # DVE + Act so both halves land in ~450ns
                                nc.scalar.copy(tt_sb[1][:, 0:256],
                                               ptt[1][:, 0:256])
                                nc.vector.tensor_copy(tt_sb[1][:, 256:512],
                                                      ptt[1][:, 256:512])

                        # row sums AFTER T^T: their ps_m WAR (previous slot's
                        # reciprocal) is long past, so these tiny waiting
                        # matmuls never clog the PE wait queue (depth 4) and
                        # stall ready T^T work.
                        pm = ps_m.tile([P, 1], F32, tag="pm")
                        for kt in range(ext):
                            nc.tensor.matmul(pm[:], esl(kt), ones_sb[:],
                                             start=(kt == 0), stop=(kt == ext - 1))
                        rc = sp.tile([P, 1], F32, tag="rc")
                        nc.vector.reciprocal(rc[:], pm[:])

                        # pa chain fully first: its scale+bias+DMA overlap the
                        # pb chain, shortening the exposed tail after the last
                        # PE matmul of the kernel.
                        pa = ps_a.tile([P, 512], F32, tag="pa")
                        pb = ps_b.tile([P, 512], F32, tag="pb")
                        ot = op.tile([P, D], BF16, tag="ot")
                        for dd in range(DT):
                            nc.tensor.matmul(pa[:], tt_sb[dd // 4][:, (dd % 4) * P:
                                             (dd % 4 + 1) * P],
                                             wv_sb[:, dd * 1024:dd * 1024 + 512],
                                             start=(dd == 0), stop=(dd == DT - 1))
                        nc.vector.scalar_tensor_tensor(
                            ot[:, 0:512], pa[:], rc[:], bv_sb[:, 0:512],
                            op0=MUL, op1=ADD)
                        nc.sync.dma_start(out=y[ls * P:(ls + 1) * P, 0:512],
                                          in_=ot[:, 0:512])
                        for dd in range(DT):
                            nc.tensor.matmul(pb[:], tt_sb[dd // 4][:, (dd % 4) * P:
                                             (dd % 4 + 1) * P],
                                             wv_sb[:, dd * 1024 + 512:
                                                   (dd + 1) * 1024],
                                             start=(dd == 0), stop=(dd == DT - 1))
                        nc.vector.scalar_tensor_tensor(
                            ot[:, 512:1024], pb[:], rc[:],
                            bv_sb[:, 512:1024], op0=MUL, op1=ADD)
                        nc.sync.dma_start(
                            out=y[ls * P:(ls + 1) * P, 512:1024],
                            in_=ot[:, 512:1024])

    nc.compile()
    return nc


def _get_nc():
    if "nc" not in _CACHE:
        _CACHE["nc"] = _build()
    return _CACHE["nc"]


def make_in_maps(X, Wq, bq, Wk, bk, Wv, bv):
    X = np.asarray(X, np.float32)
    Wq = np.asarray(Wq, np.float32)
    Wk = np.asarray(Wk, np.float32)
    Wv = np.asarray(Wv, np.float32)
    bq = np.asarray(bq, np.float32)
    bv = np.asarray(bv, np.float32)

    G = Wq @ Wk.T                                # [D, D]
    wkbq = Wk @ bq                               # [D]
    bvp = np.ascontiguousarray(np.broadcast_to(bv[None, :], (P, D)))
    # g[do, p, dd*128+c] = G[dd*128+p, do*128+c]
    g_blk = np.ascontiguousarray(
        G.reshape(DT, P, DT, P).transpose(2, 1, 0, 3).reshape(DT, P, D))
    # wv[p, dd*1024+c] = Wv[dd*128+p, c]
    wv_blk = np.ascontiguousarray(
        Wv.astype(ml_dtypes.bfloat16).reshape(DT, P, D)
        .transpose(1, 0, 2).reshape(P, DT * D))

    masks = {}
    for h in (0, 1):
        m = np.zeros((QT, 2 * P, P), np.float32)
        for s in range(QT):
            qt = QTS[h][s]
            kk = (2 * s) * P + np.arange(2 * P)[:, None]
            qq = qt * P + np.arange(P)[None, :]
            m[s] = (kk <= qq)
        # msk2[p, t*128+c] = m.reshape(16,128,128)[t, p, c]
        masks[h] = np.ascontiguousarray(
            m.reshape(QT * 2, P, P).transpose(1, 0, 2).reshape(P, QT * 2 * P)
        ).astype(ml_dtypes.bfloat16)

    in_maps = []
    for c in range(8):
        b, h = divmod(c, 2)
        Xb = X[b]
        # xkt[p, kt*1024 + dd*128 + c] = Xb.T[dd*128+p, kt*128+c]
        xkt = np.ascontiguousarray(
            Xb.T.reshape(DT, P, KT, P).transpose(1, 2, 0, 3)
            .reshape(P, KT * D))
        xq_rows = np.concatenate(
            [Xb[qt * P:(qt + 1) * P] for qt in QTS[h]], axis=0)
        # xqt[p, dd*1024+q] = xq_rows.T[dd*128+p, q]
        xqt = np.ascontiguousarray(
            xq_rows.T.reshape(DT, P, QT * P).transpose(1, 0, 2)
            .reshape(P, DT * QT * P))
        # xn[p, kt*1024+d] = Xb[kt*128+p, d]  (natural layout, bf16)
        xn = np.ascontiguousarray(
            Xb.astype(ml_dtypes.bfloat16).reshape(KT, P, D)
            .transpose(1, 0, 2).reshape(P, KT * D))
        w = Xb @ wkbq                             # [S] additive k-bias
        wbp = np.ascontiguousarray(w.reshape(KT, P).T)   # [P, KT]
        in_maps.append({
            "xqt": xqt, "xkt": xkt, "g": g_blk, "xn": xn, "wv": wv_blk,
            "wb": wbp, "bvp": bvp, "msk": masks[h],
        })
    return in_maps


def assemble(results):
    Y = np.empty((B, S, D), np.float32)
    for c in range(8):
        b, h = divmod(c, 2)
        yc = np.asarray(results[c]["y"], dtype=np.float32)
        for s in range(QT):
            qt = QTS[h][s]
            Y[b, qt * P:(qt + 1) * P, :] = yc[s * P:(s + 1) * P, :]
    return Y


def kernel(X, Wq, bq, Wk, bk, Wv, bv):
    nc = _get_nc()
    in_maps = make_in_maps(X, Wq, bq, Wk, bk, Wv, bv)
    res = run_bass_kernel_spmd(nc, in_maps, core_ids=list(range(8)))
    return assemble(res.results)
